# revision 13
# baseline (speedup 1.0000x reference)
"""Concordance CC (segment_reduce) Trainium2 Bass kernel.

Problem: y_true, y_pred [256, 65536] f32, prefix-validity mask [256, 65536] i32.
Per row: masked means/variances/covariance (ddof=1), ccc = 2*cov /
(var_t + var_p + 2*(mean_t - mean_p)); output = mean(ccc) (scalar f32).

Strategy (data parallel over B, 8 cores x 32 rows):
Every per-row statistic is an inner product over T of columns from
W = [a, b, m] with a = y_true*mask, b = y_pred*mask, m = mask:
  S2t=a.a  Stp=a.b  S1t=a.m  S2p=b.b  S1p=b.m  L=m.m
Each core computes one 96x96 Gram matrix W^T W on the TensorEngine
(PSUM-accumulated over T) for all 32 of its rows at once.

v3 (informed by v1/v2 traces):
- DMA already streams at HBM line rate; all loss is idle gaps. So: keep
  every compute engine's total busy well under the ~72us DMA stream.
- DVE interleave throughput scales with inner run length (run-8 55G/s,
  run-16 74G/s, run-32 ~89G/s elem). One 32-row group maximizes runs.
- PE has a ~30ns/instr floor: 96-col Grams (68.6ns) amortize it;
  512 matmuls/core = ~35us.
- Tail: T-blocks [16384x3, 8192, 4096, 4096] put the smallest tiles last
  (128B descriptors still ~97% of line rate); muls/copies split into
  c-halves so PE chases at half-tile granularity.
- All staging loads are SWDGE DMAs with in-flight dtype cast (f32->bf16,
  i32->bf16 verified numerically correct on HW).
"""

import numpy as np

import concourse.bass as bass
import concourse.tile as tile
from concourse import mybir
from concourse.bass_utils import run_bass_kernel_spmd

# ---------------------------------------------------------------- constants
B, T = 256, 65536
NCORES = 8
R = B // NCORES            # rows per core = 32
R2 = R                     # single row group of 32
TBS = [16384, 16384, 16384, 8192, 4096, 4096]  # T blocks, smallest last
GCOLS = 3 * R2             # 96 Gram columns: [a_0..31, b_0..31, m_0..31]

FP = mybir.dt.bfloat16     # staging + Gram operand precision (PE-native)


def split_multi_waits(nc: bass.Bass) -> int:
    """This container's walrus build accepts at most ONE sync-wait per
    instruction, but Tile's sem assignment attaches all required waits to
    the consuming instruction. Hoist the excess onto same-engine NoOps
    inserted immediately before it (sequencers execute in order, so the
    waits are still satisfied before the instruction issues)."""
    n_split = 0
    for f in nc.m.functions:
        for bb in f.blocks:
            insts = bb.instructions
            out = []
            for inst in insts:
                si = inst.sync_info
                if si is not None and si.on_wait and len(si.on_wait) > 1:
                    waits = list(si.on_wait)
                    for w in waits[:-1]:
                        nop = mybir.InstNoOp(
                            name=f"I-wsplit-{nc.next_id()}", ins=[], outs=[]
                        )
                        nop.engine = inst.engine
                        nop.sync_info = mybir.SyncInfo(on_wait=[w], on_update=[])
                        out.append(nop)
                        n_split += 1
                    inst.sync_info = mybir.SyncInfo(
                        on_wait=[waits[-1]], on_update=list(si.on_update or [])
                    )
                out.append(inst)
            bb.instructions = out
    return n_split


def build_nc() -> bass.Bass:
    nc = bass.Bass()
    yt = nc.dram_tensor("y_true", [R, T], mybir.dt.float32, kind="ExternalInput")
    yp = nc.dram_tensor("y_pred", [R, T], mybir.dt.float32, kind="ExternalInput")
    mk = nc.dram_tensor("mask", [R, T], mybir.dt.int32, kind="ExternalInput")
    gram = nc.dram_tensor("gram", [GCOLS, GCOLS], mybir.dt.float32,
                          kind="ExternalOutput")

    with tile.TileContext(nc) as tc:
        with (
            tc.tile_pool(name="gpool", bufs=2) as gpool,
            tc.tile_pool(name="stage", bufs=3) as stage,
            tc.tile_pool(name="psum", bufs=1, space="PSUM") as psum,
            tc.tile_pool(name="outp", bufs=1) as outp,
        ):
            ps = psum.tile([GCOLS, GCOLS], mybir.dt.float32)
            nblk = len(TBS)
            lo = 0
            for tb, TBcur in enumerate(TBS):
                jb = TBcur // 128  # chunk positions per row in this block
                hi = lo + TBcur
                # G is chunk-major: G[p, ci*GCOLS + k] so each matmul
                # chunk's operand G[:, ci*96:(ci+1)*96] is contiguous
                # (strided PE APs measured ~8x slower).
                g = gpool.tile([128, jb * GCOLS], FP)
                tt = stage.tile([128, R2 * jb], FP)
                tp = stage.tile([128, R2 * jb], FP)
                tm = stage.tile([128, R2 * jb], FP)

                # staging layout: tile[p, r*jb + c] = src[r, lo + p*jb + c]
                # SWDGE casts f32/i32 -> bf16 in flight.
                src = lambda h: h[0:R2, lo:hi].rearrange("r (p c) -> p r c", p=128)
                dst = lambda t_: t_[:, :].rearrange("p (r c) -> p r c", r=R2)
                # mask first: the ScalarE m-copy and both muls depend on it
                nc.gpsimd.dma_start(out=dst(tm), in_=src(mk))
                nc.gpsimd.dma_start(out=dst(tt), in_=src(yt))
                nc.gpsimd.dma_start(out=dst(tp), in_=src(yp))

                # [p][c][r] iteration: G-side inner runs are the 32
                # contiguous cols of one tensor within a chunk (strided
                # inner writes measured 4x slower; strided reads are ok).
                gv = g[:, :].rearrange("p (c k) -> p c k", k=GCOLS)
                stg = lambda t_: t_[:, :].rearrange("p (r c) -> p c r", r=R2)
                # split ops into c-halves so PE can chase at half-tile
                # granularity (the overlap tracker resolves sub-tile deps)
                jh = jb // 2
                for h in range(2):
                    cs = slice(h * jh, (h + 1) * jh)
                    ga = gv[:, cs, 0:R2]
                    gb = gv[:, cs, R2 : 2 * R2]
                    gm = gv[:, cs, 2 * R2 : 3 * R2]
                    # m-copy on the otherwise-idle ScalarE; muls on VectorE
                    # read the tensor-major tiles (independent of the copy).
                    nc.scalar.copy(out=gm, in_=stg(tm)[:, cs, :])
                    nc.vector.tensor_mul(out=ga, in0=stg(tt)[:, cs, :], in1=gm)
                    nc.vector.tensor_mul(out=gb, in0=stg(tp)[:, cs, :], in1=gm)

                for ci in range(jb):
                    w = g[:, ci * GCOLS : (ci + 1) * GCOLS]
                    nc.tensor.matmul(
                        ps[:, :],
                        lhsT=w,
                        rhs=w,
                        start=(tb == 0 and ci == 0),
                        stop=(tb == nblk - 1 and ci == jb - 1),
                    )
                lo = hi

            out_t = outp.tile([GCOLS, GCOLS], mybir.dt.float32)
            nc.vector.tensor_copy(out=out_t[:, :], in_=ps[:, :])
            nc.sync.dma_start(out=gram[:, :], in_=out_t[:, :])
    split_multi_waits(nc)
    return nc


_NC_CACHE = None


def _get_nc():
    global _NC_CACHE
    if _NC_CACHE is None:
        _NC_CACHE = build_nc()
    return _NC_CACHE


def _ccc_from_grams(grams: list[np.ndarray]) -> np.ndarray:
    idx = np.arange(R2)
    total = 0.0
    for gm_ in grams:
        g = gm_.astype(np.float64)
        s2t = g[idx, idx]
        stp = g[idx, R2 + idx]
        s1t = g[idx, 2 * R2 + idx]
        s2p = g[R2 + idx, R2 + idx]
        s1p = g[R2 + idx, 2 * R2 + idx]
        ell = g[2 * R2 + idx, 2 * R2 + idx]
        mean_t = s1t / ell
        mean_p = s1p / ell
        denom = ell - 1.0
        var_t = (s2t - s1t * s1t / ell) / denom
        var_p = (s2p - s1p * s1p / ell) / denom
        cov = (stp - s1t * s1p / ell) / denom
        ccc = 2.0 * cov / (var_t + var_p + (mean_t - mean_p) * 2.0)
        total += ccc.sum()
    return np.float32(total / B)


def kernel(y_true, y_pred, mask) -> np.ndarray:
    y_true = np.ascontiguousarray(np.asarray(y_true, dtype=np.float32))
    y_pred = np.ascontiguousarray(np.asarray(y_pred, dtype=np.float32))
    mask = np.ascontiguousarray(np.asarray(mask, dtype=np.int32))

    nc = _get_nc()
    in_maps = [
        {
            "y_true": y_true[c * R : (c + 1) * R],
            "y_pred": y_pred[c * R : (c + 1) * R],
            "mask": mask[c * R : (c + 1) * R],
        }
        for c in range(NCORES)
    ]
    res = run_bass_kernel_spmd(nc, in_maps, core_ids=list(range(NCORES)))
    grams = [res.results[c]["gram"] for c in range(NCORES)]
    return _ccc_from_grams(grams)


# revision 14
# speedup vs baseline: 1.3753x; 1.3753x over previous
"""Concordance CC (segment_reduce) Trainium2 Bass kernel.

Problem: y_true, y_pred [256, 65536] f32, prefix-validity mask [256, 65536] i32.
Per row: masked means/variances/covariance (ddof=1), ccc = 2*cov /
(var_t + var_p + 2*(mean_t - mean_p)); output = mean(ccc) (scalar f32).

Strategy (data parallel over B, 8 cores x 32 rows):
Every per-row statistic is an inner product over T of columns from
W = [a, b, m] with a = y_true*mask, b = y_pred*mask, m = mask:
  S2t=a.a  Stp=a.b  S1t=a.m  S2p=b.b  S1p=b.m  L=m.m
Each core computes one 96x96 Gram matrix W^T W on the TensorEngine
(PSUM-accumulated over T) for all 32 of its rows at once.

v3 (informed by v1/v2 traces):
- DMA already streams at HBM line rate; all loss is idle gaps. So: keep
  every compute engine's total busy well under the ~72us DMA stream.
- DVE interleave throughput scales with inner run length (run-8 55G/s,
  run-16 74G/s, run-32 ~89G/s elem). One 32-row group maximizes runs.
- PE has a ~30ns/instr floor: 96-col Grams (68.6ns) amortize it;
  512 matmuls/core = ~35us.
- Tail: T-blocks [16384x3, 8192, 4096, 4096] put the smallest tiles last
  (128B descriptors still ~97% of line rate); muls/copies split into
  c-halves so PE chases at half-tile granularity.
- All staging loads are SWDGE DMAs with in-flight dtype cast (f32->bf16,
  i32->bf16 verified numerically correct on HW).
"""

import numpy as np

import concourse.bass as bass
import concourse.tile as tile
from concourse import mybir
from concourse.bass_utils import run_bass_kernel_spmd

# ---------------------------------------------------------------- constants
B, T = 256, 65536
NCORES = 8
R = B // NCORES            # rows per core = 32
R2 = R                     # single row group of 32
TBS = [16384, 16384, 16384, 8192, 4096, 4096]  # T blocks, smallest last
GCOLS = 3 * R2             # 96 Gram columns: [a_0..31, b_0..31, m_0..31]

FP = mybir.dt.bfloat16     # staging + Gram operand precision (PE-native)


def split_multi_waits(nc: bass.Bass) -> int:
    """This container's walrus build accepts at most ONE sync-wait per
    instruction, but Tile's sem assignment attaches all required waits to
    the consuming instruction. Hoist the excess onto same-engine NoOps
    inserted immediately before it (sequencers execute in order, so the
    waits are still satisfied before the instruction issues)."""
    n_split = 0
    for f in nc.m.functions:
        for bb in f.blocks:
            insts = bb.instructions
            out = []
            for inst in insts:
                si = inst.sync_info
                if si is not None and si.on_wait and len(si.on_wait) > 1:
                    waits = list(si.on_wait)
                    for w in waits[:-1]:
                        nop = mybir.InstNoOp(
                            name=f"I-wsplit-{nc.next_id()}", ins=[], outs=[]
                        )
                        nop.engine = inst.engine
                        nop.sync_info = mybir.SyncInfo(on_wait=[w], on_update=[])
                        out.append(nop)
                        n_split += 1
                    inst.sync_info = mybir.SyncInfo(
                        on_wait=[waits[-1]], on_update=list(si.on_update or [])
                    )
                out.append(inst)
            bb.instructions = out
    return n_split


def build_nc() -> bass.Bass:
    nc = bass.Bass()
    yt = nc.dram_tensor("y_true", [R, T], mybir.dt.float32, kind="ExternalInput")
    yp = nc.dram_tensor("y_pred", [R, T], mybir.dt.float32, kind="ExternalInput")
    mk = nc.dram_tensor("mask", [R, T], mybir.dt.int32, kind="ExternalInput")
    gram = nc.dram_tensor("gram", [GCOLS, GCOLS], mybir.dt.float32,
                          kind="ExternalOutput")

    with tile.TileContext(nc) as tc:
        with (
            tc.tile_pool(name="gpool", bufs=2) as gpool,
            tc.tile_pool(name="stage", bufs=3) as stage,
            tc.tile_pool(name="psum", bufs=1, space="PSUM") as psum,
            tc.tile_pool(name="outp", bufs=1) as outp,
        ):
            ps = psum.tile([GCOLS, GCOLS], mybir.dt.float32)
            nblk = len(TBS)
            lo = 0
            for tb, TBcur in enumerate(TBS):
                jb = TBcur // 128  # chunk positions per row in this block
                hi = lo + TBcur
                # G is chunk-major: G[p, ci*GCOLS + k] so each matmul
                # chunk's operand G[:, ci*96:(ci+1)*96] is contiguous
                # (strided PE APs measured ~8x slower).
                g = gpool.tile([128, jb * GCOLS], FP)
                tt = stage.tile([128, R2 * jb], FP)
                tp = stage.tile([128, R2 * jb], FP)
                tm = stage.tile([128, R2 * jb], FP)

                # staging layout: tile[p, r*jb + c] = src[r, lo + p*jb + c]
                # SWDGE casts f32/i32 -> bf16 in flight.
                src = lambda h: h[0:R2, lo:hi].rearrange("r (p c) -> p r c", p=128)
                dst = lambda t_: t_[:, :].rearrange("p (r c) -> p r c", r=R2)
                # mask first: the ScalarE m-copy and both muls depend on it
                nc.gpsimd.dma_start(out=dst(tm), in_=src(mk))
                nc.gpsimd.dma_start(out=dst(tt), in_=src(yt))
                nc.gpsimd.dma_start(out=dst(tp), in_=src(yp))

                # [p][c][r] iteration: G-side inner runs are the 32
                # contiguous cols of one tensor within a chunk (strided
                # inner writes measured 4x slower; strided reads are ok).
                gv = g[:, :].rearrange("p (c k) -> p c k", k=GCOLS)
                stg = lambda t_: t_[:, :].rearrange("p (r c) -> p c r", r=R2)
                # split ops into c-halves so PE can chase at half-tile
                # granularity (the overlap tracker resolves sub-tile deps)
                jh = jb // 2
                for h in range(2):
                    cs = slice(h * jh, (h + 1) * jh)
                    ga = gv[:, cs, 0:R2]
                    gb = gv[:, cs, R2 : 2 * R2]
                    gm = gv[:, cs, 2 * R2 : 3 * R2]
                    # m-copy on the otherwise-idle ScalarE; muls on VectorE
                    # read the tensor-major tiles (independent of the copy).
                    nc.scalar.copy(out=gm, in_=stg(tm)[:, cs, :])
                    nc.vector.tensor_mul(out=ga, in0=stg(tt)[:, cs, :],
                                         in1=stg(tm)[:, cs, :])
                    nc.vector.tensor_mul(out=gb, in0=stg(tp)[:, cs, :],
                                         in1=stg(tm)[:, cs, :])

                for ci in range(jb):
                    w = g[:, ci * GCOLS : (ci + 1) * GCOLS]
                    nc.tensor.matmul(
                        ps[:, :],
                        lhsT=w,
                        rhs=w,
                        start=(tb == 0 and ci == 0),
                        stop=(tb == nblk - 1 and ci == jb - 1),
                    )
                lo = hi

            out_t = outp.tile([GCOLS, GCOLS], mybir.dt.float32)
            nc.vector.tensor_copy(out=out_t[:, :], in_=ps[:, :])
            nc.sync.dma_start(out=gram[:, :], in_=out_t[:, :])
    split_multi_waits(nc)
    return nc


_NC_CACHE = None


def _get_nc():
    global _NC_CACHE
    if _NC_CACHE is None:
        _NC_CACHE = build_nc()
    return _NC_CACHE


def _ccc_from_grams(grams: list[np.ndarray]) -> np.ndarray:
    idx = np.arange(R2)
    total = 0.0
    for gm_ in grams:
        g = gm_.astype(np.float64)
        s2t = g[idx, idx]
        stp = g[idx, R2 + idx]
        s1t = g[idx, 2 * R2 + idx]
        s2p = g[R2 + idx, R2 + idx]
        s1p = g[R2 + idx, 2 * R2 + idx]
        ell = g[2 * R2 + idx, 2 * R2 + idx]
        mean_t = s1t / ell
        mean_p = s1p / ell
        denom = ell - 1.0
        var_t = (s2t - s1t * s1t / ell) / denom
        var_p = (s2p - s1p * s1p / ell) / denom
        cov = (stp - s1t * s1p / ell) / denom
        ccc = 2.0 * cov / (var_t + var_p + (mean_t - mean_p) * 2.0)
        total += ccc.sum()
    return np.float32(total / B)


def kernel(y_true, y_pred, mask) -> np.ndarray:
    y_true = np.ascontiguousarray(np.asarray(y_true, dtype=np.float32))
    y_pred = np.ascontiguousarray(np.asarray(y_pred, dtype=np.float32))
    mask = np.ascontiguousarray(np.asarray(mask, dtype=np.int32))

    nc = _get_nc()
    in_maps = [
        {
            "y_true": y_true[c * R : (c + 1) * R],
            "y_pred": y_pred[c * R : (c + 1) * R],
            "mask": mask[c * R : (c + 1) * R],
        }
        for c in range(NCORES)
    ]
    res = run_bass_kernel_spmd(nc, in_maps, core_ids=list(range(NCORES)))
    grams = [res.results[c]["gram"] for c in range(NCORES)]
    return _ccc_from_grams(grams)


# revision 15
# speedup vs baseline: 1.4050x; 1.0216x over previous
"""Concordance CC (segment_reduce) Trainium2 Bass kernel.

Problem: y_true, y_pred [256, 65536] f32, prefix-validity mask [256, 65536] i32.
Per row: masked means/variances/covariance (ddof=1), ccc = 2*cov /
(var_t + var_p + 2*(mean_t - mean_p)); output = mean(ccc) (scalar f32).

Strategy (data parallel over B, 8 cores x 32 rows):
Every per-row statistic is an inner product over T of columns from
W = [a, b, m] with a = y_true*mask, b = y_pred*mask, m = mask:
  S2t=a.a  Stp=a.b  S1t=a.m  S2p=b.b  S1p=b.m  L=m.m
Each core computes one 96x96 Gram matrix W^T W on the TensorEngine
(PSUM-accumulated over T) for all 32 of its rows at once.

v9 (informed by v1-v8 traces):
- The DMA stream wall (start ~11.4us + ~83us) is the span governor; DMA busy
  tracks descriptor count (rows x 128 per dma_start), so blocks are merged to
  [16K,16K,16K,12K,4K] = 61440 descriptors (was 73728), smallest block last
  so the post-stream compute tail stays ~3us.
- Pipeline fill: block 0's three loads are split into row-halves (6 DMAs) and
  its muls/copies split likewise, so the first DVE mul starts at ~15.5us
  instead of 24.5us without changing the steady-state mul configuration.
- DVE tensor_mul runs at ~56-60G elem/s on this AP shape (two strided
  walkers); both muls read the tensor-major tiles so they depend only on
  DMAs, never on the ScalarE m-copy (in1=gm variants measured bimodal and
  occasionally pathological).
- All loads are SWDGE DMAs with in-flight dtype cast (f32/i32 -> bf16,
  verified numerically correct on HW). PE: 96-col Gram chunks (68.6ns)
  amortize the ~30ns matmul floor; c-halved muls let PE chase sub-tile.
"""

import numpy as np

import concourse.bass as bass
import concourse.tile as tile
from concourse import mybir
from concourse.bass_utils import run_bass_kernel_spmd

# ---------------------------------------------------------------- constants
B, T = 256, 65536
NCORES = 8
R = B // NCORES            # rows per core = 32
R2 = R                     # single row group of 32
TBS = [16384, 16384, 16384, 12288, 4096]  # merged blocks, smallest last
GCOLS = 3 * R2             # 96 Gram columns: [a_0..31, b_0..31, m_0..31]

FP = mybir.dt.bfloat16     # staging + Gram operand precision (PE-native)


def split_multi_waits(nc: bass.Bass) -> int:
    """This container's walrus build accepts at most ONE sync-wait per
    instruction, but Tile's sem assignment attaches all required waits to
    the consuming instruction. Hoist the excess onto same-engine NoOps
    inserted immediately before it (sequencers execute in order, so the
    waits are still satisfied before the instruction issues)."""
    n_split = 0
    for f in nc.m.functions:
        for bb in f.blocks:
            insts = bb.instructions
            out = []
            for inst in insts:
                si = inst.sync_info
                if si is not None and si.on_wait and len(si.on_wait) > 1:
                    waits = list(si.on_wait)
                    for w in waits[:-1]:
                        nop = mybir.InstNoOp(
                            name=f"I-wsplit-{nc.next_id()}", ins=[], outs=[]
                        )
                        nop.engine = inst.engine
                        nop.sync_info = mybir.SyncInfo(on_wait=[w], on_update=[])
                        out.append(nop)
                        n_split += 1
                    inst.sync_info = mybir.SyncInfo(
                        on_wait=[waits[-1]], on_update=list(si.on_update or [])
                    )
                out.append(inst)
            bb.instructions = out
    return n_split


def build_nc() -> bass.Bass:
    nc = bass.Bass()
    yt = nc.dram_tensor("y_true", [R, T], mybir.dt.float32, kind="ExternalInput")
    yp = nc.dram_tensor("y_pred", [R, T], mybir.dt.float32, kind="ExternalInput")
    mk = nc.dram_tensor("mask", [R, T], mybir.dt.int32, kind="ExternalInput")
    gram = nc.dram_tensor("gram", [GCOLS, GCOLS], mybir.dt.float32,
                          kind="ExternalOutput")

    with tile.TileContext(nc) as tc:
        with (
            tc.tile_pool(name="gpool", bufs=2) as gpool,
            tc.tile_pool(name="stage", bufs=3) as stage,
            tc.tile_pool(name="psum", bufs=1, space="PSUM") as psum,
            tc.tile_pool(name="outp", bufs=1) as outp,
        ):
            ps = psum.tile([GCOLS, GCOLS], mybir.dt.float32)
            nblk = len(TBS)
            lo = 0
            for tb, TBcur in enumerate(TBS):
                jb = TBcur // 128  # chunk positions per row in this block
                hi = lo + TBcur
                # G is chunk-major: G[p, ci*GCOLS + k] so each matmul
                # chunk's operand G[:, ci*96:(ci+1)*96] is contiguous
                # (strided PE APs measured ~8x slower).
                g = gpool.tile([128, jb * GCOLS], FP)
                tt = stage.tile([128, R2 * jb], FP)
                tp = stage.tile([128, R2 * jb], FP)
                tm = stage.tile([128, R2 * jb], FP)

                # staging layout: tile[p, r*jb + c] = src[r, lo + p*jb + c]
                # SWDGE casts f32/i32 -> bf16 in flight.
                # Block 0: row-halved loads so the first mul starts as soon
                # as the first 16 rows of mask+y_true have landed.
                row_splits = [(0, 16), (16, 32)] if tb == 0 else [(0, 32)]
                for (ra, rb) in row_splits:
                    nr = rb - ra
                    src = lambda h: h[ra:rb, lo:hi].rearrange(
                        "r (p c) -> p r c", p=128
                    )
                    dst = lambda t_: t_[:, ra * jb : rb * jb].rearrange(
                        "p (r c) -> p r c", r=nr
                    )
                    # mask first: the ScalarE m-copy and both muls depend on it
                    nc.gpsimd.dma_start(out=dst(tm), in_=src(mk))
                    nc.gpsimd.dma_start(out=dst(tt), in_=src(yt))
                    nc.gpsimd.dma_start(out=dst(tp), in_=src(yp))

                # [p][c][r] iteration: G-side inner runs are the contiguous
                # cols of one tensor within a chunk (strided inner writes
                # measured 4x slower; strided reads are ok).
                gv = g[:, :].rearrange("p (c k) -> p c k", k=GCOLS)
                # split ops into c-halves so PE can chase at half-tile
                # granularity (the overlap tracker resolves sub-tile deps);
                # block 0 additionally splits by row-halves to match its DMAs.
                jh = jb // 2
                for (ra, rb) in row_splits:
                    nr = rb - ra
                    stg = lambda t_: t_[:, ra * jb : rb * jb].rearrange(
                        "p (r c) -> p c r", r=nr
                    )
                    for h in range(2):
                        cs = slice(h * jh, (h + 1) * jh)
                        ga = gv[:, cs, ra:rb]
                        gb = gv[:, cs, R2 + ra : R2 + rb]
                        gm = gv[:, cs, 2 * R2 + ra : 2 * R2 + rb]
                        # m-copy on the otherwise-idle ScalarE; muls on
                        # VectorE read the tensor-major tiles (independent
                        # of the copy).
                        nc.scalar.copy(out=gm, in_=stg(tm)[:, cs, :])
                        nc.vector.tensor_mul(out=ga, in0=stg(tt)[:, cs, :],
                                             in1=stg(tm)[:, cs, :])
                        nc.vector.tensor_mul(out=gb, in0=stg(tp)[:, cs, :],
                                             in1=stg(tm)[:, cs, :])

                for ci in range(jb):
                    w = g[:, ci * GCOLS : (ci + 1) * GCOLS]
                    nc.tensor.matmul(
                        ps[:, :],
                        lhsT=w,
                        rhs=w,
                        start=(tb == 0 and ci == 0),
                        stop=(tb == nblk - 1 and ci == jb - 1),
                    )
                lo = hi

            out_t = outp.tile([GCOLS, GCOLS], mybir.dt.float32)
            nc.vector.tensor_copy(out=out_t[:, :], in_=ps[:, :])
            nc.sync.dma_start(out=gram[:, :], in_=out_t[:, :])
    split_multi_waits(nc)
    return nc


_NC_CACHE = None


def _get_nc():
    global _NC_CACHE
    if _NC_CACHE is None:
        _NC_CACHE = build_nc()
    return _NC_CACHE


def _ccc_from_grams(grams: list[np.ndarray]) -> np.ndarray:
    idx = np.arange(R2)
    total = 0.0
    for gm_ in grams:
        g = gm_.astype(np.float64)
        s2t = g[idx, idx]
        stp = g[idx, R2 + idx]
        s1t = g[idx, 2 * R2 + idx]
        s2p = g[R2 + idx, R2 + idx]
        s1p = g[R2 + idx, 2 * R2 + idx]
        ell = g[2 * R2 + idx, 2 * R2 + idx]
        mean_t = s1t / ell
        mean_p = s1p / ell
        denom = ell - 1.0
        var_t = (s2t - s1t * s1t / ell) / denom
        var_p = (s2p - s1p * s1p / ell) / denom
        cov = (stp - s1t * s1p / ell) / denom
        ccc = 2.0 * cov / (var_t + var_p + (mean_t - mean_p) * 2.0)
        total += ccc.sum()
    return np.float32(total / B)


def kernel(y_true, y_pred, mask) -> np.ndarray:
    y_true = np.ascontiguousarray(np.asarray(y_true, dtype=np.float32))
    y_pred = np.ascontiguousarray(np.asarray(y_pred, dtype=np.float32))
    mask = np.ascontiguousarray(np.asarray(mask, dtype=np.int32))

    nc = _get_nc()
    in_maps = [
        {
            "y_true": y_true[c * R : (c + 1) * R],
            "y_pred": y_pred[c * R : (c + 1) * R],
            "mask": mask[c * R : (c + 1) * R],
        }
        for c in range(NCORES)
    ]
    res = run_bass_kernel_spmd(nc, in_maps, core_ids=list(range(NCORES)))
    grams = [res.results[c]["gram"] for c in range(NCORES)]
    return _ccc_from_grams(grams)
